# revision 10
# baseline (speedup 1.0000x reference)
"""AttentionBlock (GroupNorm + single-head self-attention + proj + residual)
for Trainium2, distributed over 8 NeuronCores.

Sharding: data-parallel over batch B=4 (2 cores per batch) x sequence-parallel
over the 4096 tokens (each core handles 2048 query tokens, full keys/values).
Per-core inputs are column-permuted so each core's query half sits in columns
[0, 2048) -- attention/GroupNorm are permutation-invariant over key columns.

All heavy matmuls run in fp8e4m3 with the DoubleRow perf mode. x ships from
the host pre-cast to fp8 (GroupNorm stats tolerate the quantization) plus an
fp32 query-half for the residual. GroupNorm is folded into the QKV weights
(host pre-folds gn_w and the x16 fp8 scale into bf16 weights; the device
multiplies by the runtime rstd only). The k bias drops entirely (softmax rows
are invariant to per-query constants) and the v bias rides through softmax
into the proj/residual bias.

v2 vs v1:
- Head: 4 DMA queues with x8 strictly first, bf16 weights (half the const
  bytes), GN stats split across DVE (12x bn_stats) + ACT (copy-accum sums) +
  GpSimd (square for sum-of-squares), and dummy PE matmuls to hold the HAM
  clock-gate open so the real stream starts warm.
- Rowsums use a 1-column all-ones stationary ([1,512] output) instead of a
  128-col broadcast stationary: kills 64 256-col LDWEIGHTS (the PE steady
  state is LDWEIGHTS-bound at ~213ns each). The reciprocal is broadcast back
  to 128 partitions with one tiny fp16 matmul per query group.
- k casts moved off the ACT engine (it is ~75% busy with exp); xqp built on
  GpSimd.
- proj emitted as both halves back-to-back (less psS pool contention); final
  group epilogue split into quarters across 4 DMA queues.
"""
import sys

sys.path.insert(0, "/opt/trn_rl_repo")

import ml_dtypes
import numpy as np

import concourse.bass as bass
import concourse.mybir as mybir
import concourse.tile as tile
from concourse import bacc
from concourse.bass_utils import run_bass_kernel_spmd

F32 = mybir.dt.float32
BF16 = mybir.dt.bfloat16
F16 = mybir.dt.float16
F8 = mybir.dt.float8e4
AF = mybir.ActivationFunctionType
DR = mybir.MatmulPerfMode.DoubleRow
ALU = mybir.AluOpType

B, C, HW = 4, 256, 4096          # batch, channels, tokens per image
G = 8                            # groupnorm groups
NCORES = 8
NQ = HW // 2                     # query tokens per core (2048)
QG = 512                         # query-group width (columns per softmax pass)
NGROUPS = NQ // QG               # 4 query groups per core
NPAIR = HW // 256                # 16 key-pair tiles of 256 tokens
EPS = 1e-5
NWARM = 56                       # dummy PE matmuls to hold HAM warm pre-loop

# bf16 consts layout: [128, 2048] = Whqkv (1536) | Whproj (512)
OFF_WQKV = 0
OFF_WPROJ = 1536
NCBF = 2048
# f32 consts layout: [128, 134] = grpavg (128) | cq (2) | cv (2) | cp (2)
OFF_GRPAVG = 0
OFF_CQ = 128
OFF_CV = 130
OFF_CP = 132
NCF32 = 134


def _build_nc(debug=False):
    nc = bacc.Bacc("TRN2")

    x8in = nc.dram_tensor("x8in", [128, 2 * HW], F8, kind="ExternalInput")
    xq = nc.dram_tensor("xq", [C, NQ], F32, kind="ExternalInput")
    cbf = nc.dram_tensor("cbf", [128, NCBF], BF16, kind="ExternalInput")
    cf32 = nc.dram_tensor("cf32", [128, NCF32], F32, kind="ExternalInput")
    out = nc.dram_tensor("out", [C, NQ], F32, kind="ExternalOutput")
    if debug:
        dbg = {
            "d_q8": nc.dram_tensor("d_q8", [128, 2 * NQ], F8, kind="ExternalOutput"),
            "d_k8": nc.dram_tensor("d_k8", [128, 2 * HW], F8, kind="ExternalOutput"),
            "d_v8": nc.dram_tensor("d_v8", [128, 2 * HW], F8, kind="ExternalOutput"),
            "d_w8": nc.dram_tensor("d_w8", [128, 1536], F8, kind="ExternalOutput"),
            "d_rstd": nc.dram_tensor("d_rstd", [128, 2], F32, kind="ExternalOutput"),
            "d_mv": nc.dram_tensor("d_mv", [128, 4], F32, kind="ExternalOutput"),
            "d_og": nc.dram_tensor("d_og", [128, 2 * QG], F8, kind="ExternalOutput"),
            "d_rb": nc.dram_tensor("d_rb", [1, QG], F32, kind="ExternalOutput"),
        }

    with tile.TileContext(nc) as tc:
        with (
            tc.tile_pool(name="big", bufs=1) as big,       # long-lived big tensors
            tc.tile_pool(name="small", bufs=1) as small,   # weights, vectors
            tc.tile_pool(name="pt", bufs=8) as ptp,        # exp(scores) fp8 pairs
            tc.tile_pool(name="og", bufs=3) as ogp,        # normalized attn out fp8
            tc.tile_pool(name="rb", bufs=2) as rbp,        # reciprocal rowsums
            tc.tile_pool(name="tmp", bufs=4) as tmpp,      # small working tiles
            tc.tile_pool(name="t1", bufs=3) as t1p,        # proj epilogue staging
            tc.tile_pool(name="xqp", bufs=2) as xqpp,      # x + pb_tot per group
            tc.tile_pool(name="psS", bufs=2, space="PSUM") as psS,   # scores/qkv/proj
            tc.tile_pool(name="psO", bufs=2, space="PSUM") as psO,   # attn out accum
            tc.tile_pool(name="psU", bufs=2, space="PSUM") as psU,   # rowsums + small
        ):
            # ---------------- input DMA, deadline-ordered -----------------
            # x8 strictly first across 4 engine queues (sync, tensor, scalar,
            # gpsimd); weights/residual stream in behind it.
            x8 = big.tile([128, 2 * HW], F8, tag="x8")
            x8v = x8.rearrange("p (a n) -> p a n", a=2)
            cbf_sb = big.tile([128, NCBF], BF16, tag="cbf")
            cf_sb = big.tile([128, NCF32], F32, tag="cf32")
            xq_sb = big.tile([128, 2 * NQ], F32, tag="xq")

            dmaq3 = [nc.sync, nc.scalar, nc.gpsimd]
            for p in range(8):  # 1024-col chunks (2 bn_stats units each)
                dmaq3[p % 3].dma_start(
                    out=x8[:, p * 1024 : (p + 1) * 1024],
                    in_=x8in[:, p * 1024 : (p + 1) * 1024],
                )
            nc.gpsimd.dma_start(out=cf_sb, in_=cf32[:, :])
            nc.sync.dma_start(out=cbf_sb[:, 0:1024], in_=cbf[:, 0:1024])
            nc.scalar.dma_start(out=cbf_sb[:, 1024:2048], in_=cbf[:, 1024:2048])
            for cb in range(2):
                dmaq = nc.gpsimd if cb == 0 else nc.sync
                dmaq.dma_start(
                    out=xq_sb[:, cb * NQ : (cb + 1) * NQ],
                    in_=xq[cb * 128 : (cb + 1) * 128, :],
                )

            # ---------------- constants / views ----------------
            wqkv_bf = cbf_sb[:, OFF_WQKV : OFF_WQKV + 1536]
            wproj_bf = cbf_sb[:, OFF_WPROJ : OFF_WPROJ + 512]
            grpavg_sb = cf_sb[:, OFF_GRPAVG : OFF_GRPAVG + 128]
            cq = [cf_sb[:, OFF_CQ + o : OFF_CQ + 1 + o] for o in range(2)]
            cv = [cf_sb[:, OFF_CV + o : OFF_CV + 1 + o] for o in range(2)]
            cp = [cf_sb[:, OFF_CP + o : OFF_CP + 1 + o] for o in range(2)]

            eps_t = small.tile([128, 1], F32, tag="eps")
            nc.vector.memset(eps_t, EPS)
            expb_t = small.tile([128, 1], F32, tag="expb")
            nc.vector.memset(expb_t, -3.0)
            zero_t = small.tile([128, 1], F32, tag="zero")
            nc.vector.memset(zero_t, 0.0)
            # all-ones fp8, [128, 32] so the [p, 2, 1] DR view has 16B a-stride
            ones8 = small.tile([128, 32], F8, tag="ones8")
            nc.vector.memset(ones8, 1.0)
            ones8v1 = ones8.rearrange("p (a b) -> p a b", a=2)[:, :, 0:1]
            # fp16 ones row for the rowsum broadcast matmul
            ones16 = small.tile([1, 128], F16, tag="ones16")
            nc.vector.memset(ones16, 1.0)

            # preload sqrt+exp ACT tables during the DMA wait
            warm_t = small.tile([128, 1], F32, tag="warm")
            nc.scalar.activation(out=warm_t, in_=eps_t, func=AF.Sqrt, bias=zero_t)
            nc.scalar.activation(out=warm_t, in_=eps_t, func=AF.Exp, bias=zero_t)

            # ---------------- PE warm-up dummies ----------------
            # Keep the HAM activity monitor busy from first-x8-arrival until
            # the real stream starts, so real matmuls run at K=8/8 (2.4GHz).
            for i in range(NWARM):
                wm = psS.tile([1, 512], F32, tag="s", name=f"warm{i}")
                nc.tensor.matmul(
                    wm, ones8v1, x8v[:, :, 0:512], start=True, stop=True,
                    perf_mode=DR,
                )

            # ---------------- GN stats, 3-way split ----------------
            # DVE: bn_stats on chunks c0-c5 (12x512 units, streamed per DMA).
            # ACT: copy-accum sum over c6+c7; GpSimd: square for the sumsq,
            # ACT copy-accums it. (gpsimd has no PSUM port but SBUF is fine)
            stats0 = tmpp.tile([128, 8, 6], F32, tag="bnstats0", name="bnstats0")
            stats1 = tmpp.tile([128, 4, 6], F32, tag="bnstats1", name="bnstats1")
            for u in range(8):
                nc.vector.bn_stats(
                    out=stats0[:, u, :], in_=x8v[:, 0, u * 512 : (u + 1) * 512]
                )
            for u in range(4):
                nc.vector.bn_stats(
                    out=stats1[:, u, :], in_=x8v[:, 1, u * 512 : (u + 1) * 512]
                )
            tailx = x8v[:, 1, 2048:4096]                    # [128, 2048]
            trash8 = big.tile([128, 2048], F8, tag="trash8")
            sumb = tmpp.tile([128, 1], F32, tag="sumb")
            nc.scalar.activation(
                out=trash8, in_=tailx, func=AF.Copy, accum_out=sumb
            )
            sq = big.tile([128, 2048], F32, tag="sq")
            nc.gpsimd.tensor_mul(out=sq, in0=tailx, in1=tailx)
            trash8b = trash8.bitcast(F8)  # reuse scratch for the 2nd accum
            ssb = tmpp.tile([128, 1], F32, tag="ssb")
            nc.scalar.activation(
                out=trash8b, in_=sq, func=AF.Copy, accum_out=ssb
            )

            # ---------------- per-channel (mean, E2) ----------------
            mv = tmpp.tile([128, 2, 2], F32, tag="mv")  # [cb][mean, E2]
            nc.vector.bn_aggr(out=mv[:, 0, :], in_=stats0)
            # E2 = mean^2 + var  (in place on the var slot)
            nc.vector.scalar_tensor_tensor(
                out=mv[:, 0, 1:2], in0=mv[:, 0, 0:1], scalar=mv[:, 0, 0:1],
                in1=mv[:, 0, 1:2], op0=ALU.mult, op1=ALU.add,
            )
            mv1a = tmpp.tile([128, 2], F32, tag="mv1a")
            nc.vector.bn_aggr(out=mv1a, in_=stats1)
            nc.vector.scalar_tensor_tensor(
                out=mv1a[:, 1:2], in0=mv1a[:, 0:1], scalar=mv1a[:, 0:1],
                in1=mv1a[:, 1:2], op0=ALU.mult, op1=ALU.add,
            )
            # combine the bn_aggr half (2048 els) with the raw-sum half:
            # mean = mu_a/2 + sumb/4096 ; E2 = E2_a/2 + ssb/4096
            tmpc = tmpp.tile([128, 2], F32, tag="tmpc")
            nc.vector.tensor_scalar_mul(out=tmpc[:, 0:1], in0=sumb, scalar1=1.0 / 4096.0)
            nc.vector.tensor_scalar_mul(out=tmpc[:, 1:2], in0=ssb, scalar1=1.0 / 4096.0)
            for j in range(2):
                nc.vector.scalar_tensor_tensor(
                    out=mv[:, 1, j : j + 1], in0=mv1a[:, j : j + 1], scalar=0.5,
                    in1=tmpc[:, j : j + 1], op0=ALU.mult, op1=ALU.add,
                )

            # ---------------- group averaging + rstd ----------------
            cst_ps = psU.tile([128, 4], F32, tag="u", name="cst")
            for cb in range(2):
                nc.tensor.matmul(
                    cst_ps[:, 2 * cb : 2 * cb + 2], grpavg_sb, mv[:, cb, :],
                    start=True, stop=True,
                )
            cst = tmpp.tile([128, 4], F32, tag="cst")
            nc.vector.tensor_copy(out=cst, in_=cst_ps)
            cstv = cst.rearrange("p (a b) -> p a b", b=2)
            mu2 = cstv[:, :, 0]       # [128, 2] group means
            negvar = tmpp.tile([128, 2], F32, tag="negvar")
            for cb in range(2):
                nc.vector.scalar_tensor_tensor(
                    out=negvar[:, cb : cb + 1], in0=cst[:, 2 * cb : 2 * cb + 1],
                    scalar=cst[:, 2 * cb : 2 * cb + 1],
                    in1=cst[:, 2 * cb + 1 : 2 * cb + 2],
                    op0=ALU.mult, op1=ALU.subtract,
                )
            rstd = tmpp.tile([128, 2], F32, tag="rstd")
            nc.scalar.activation(
                out=rstd, in_=negvar, func=AF.Sqrt, bias=eps_t, scale=-1.0
            )
            nc.vector.reciprocal(out=rstd, in_=rstd)

            # ---------------- fold rstd into fp8 weights ----------------
            w8 = small.tile([128, 1536], F8, tag="w8")
            for cb in range(2):
                nc.vector.tensor_scalar_mul(
                    out=w8[:, cb * 768 : (cb + 1) * 768],
                    in0=wqkv_bf[:, cb * 768 : (cb + 1) * 768],
                    scalar1=rstd[:, cb : cb + 1],
                )
            w8v = w8.rearrange("p (a o) -> p a o", a=2)
            wp8 = small.tile([128, 512], F8, tag="wp8")
            nc.vector.tensor_copy(out=wp8, in_=wproj_bf)
            wp8v = wp8.rearrange("p (a o) -> p a o", a=2)

            # m = mu_g * rstd_g (bf16) feeds all bias matvecs
            mbf = tmpp.tile([128, 2], BF16, tag="mbf")
            nc.vector.tensor_mul(out=mbf, in0=mu2, in1=rstd)

            # q bias: bq = cq - (1/16) Whq^T m
            bq = small.tile([128, 2], F32, tag="bq")
            for ob in range(2):
                bq_ps = psU.tile([128, 1], F32, tag="u", name=f"bq{ob}")
                for cb in range(2):
                    nc.tensor.matmul(
                        bq_ps,
                        wqkv_bf[:, cb * 768 + ob * 128 : cb * 768 + (ob + 1) * 128],
                        mbf[:, cb : cb + 1],
                        start=(cb == 0), stop=(cb == 1),
                    )
                nc.vector.tensor_scalar(
                    out=bq[:, ob : ob + 1], in0=bq_ps, scalar1=-1.0 / 16.0,
                    scalar2=cq[ob], op0=ALU.mult, op1=ALU.add,
                )
            # v bias rides through softmax into the proj bias (deferred)
            vbt = small.tile([128, 2], BF16, tag="vbt")
            pbt = small.tile([128, 2], F32, tag="pbt")

            def emit_pbt():
                for vbk in range(2):
                    bv_ps = psU.tile([128, 1], F32, tag="u", name=f"bv{vbk}")
                    for cb in range(2):
                        nc.tensor.matmul(
                            bv_ps,
                            wqkv_bf[:, cb * 768 + 512 + vbk * 128 : cb * 768 + 512 + (vbk + 1) * 128],
                            mbf[:, cb : cb + 1],
                            start=(cb == 0), stop=(cb == 1),
                        )
                    nc.vector.tensor_scalar(
                        out=vbt[:, vbk : vbk + 1], in0=bv_ps, scalar1=-1.0 / 16.0,
                        scalar2=cv[vbk], op0=ALU.mult, op1=ALU.add,
                    )
                for pbk in range(2):
                    pp_ps = psU.tile([128, 1], F32, tag="u", name=f"pbs{pbk}")
                    for cb in range(2):
                        nc.tensor.matmul(
                            pp_ps,
                            wproj_bf[:, cb * 256 + pbk * 128 : cb * 256 + (pbk + 1) * 128],
                            vbt[:, cb : cb + 1],
                            start=(cb == 0), stop=(cb == 1),
                        )
                    nc.vector.tensor_scalar(
                        out=pbt[:, pbk : pbk + 1], in0=pp_ps, scalar1=1.0 / 16.0,
                        scalar2=cp[pbk], op0=ALU.mult, op1=ALU.add,
                    )

            # ---------------- QKV production (fp8, DoubleRow) ----------------
            q8 = big.tile([128, 2 * NQ], F8, tag="q8")
            q8v = q8.rearrange("p (a n) -> p a n", a=2)
            k8 = big.tile([128, 2 * HW], F8, tag="k8")
            k8v = k8.rearrange("p (a n) -> p a n", a=2)
            vT8 = big.tile([128, 2 * HW], F8, tag="vT8")

            def emit_q(g):
                ps = psS.tile([128, 1024], F32, tag="s", name=f"qp{g}")
                qs = slice(g * QG, (g + 1) * QG)
                for ob in range(2):
                    nc.tensor.matmul(
                        ps[:, ob * 512 : (ob + 1) * 512],
                        w8v[:, :, ob * 128 : (ob + 1) * 128],
                        x8v[:, :, qs],
                        start=True, stop=True, perf_mode=DR,
                    )
                    nc.vector.tensor_scalar(
                        out=q8v[:, ob, qs],
                        in0=ps[:, ob * 512 : (ob + 1) * 512],
                        scalar1=1.0 / 16.0,
                        scalar2=bq[:, ob : ob + 1],
                        op0=ALU.mult, op1=ALU.add,
                    )

            def emit_k(kc):
                # k for 512-token chunk kc; bias drops (softmax rows are
                # invariant to per-query constants); cast on DVE
                ps = psS.tile([128, 1024], F32, tag="s", name=f"kp{kc}")
                ts = slice(kc * 512, (kc + 1) * 512)
                for ob in range(2):
                    nc.tensor.matmul(
                        ps[:, ob * 512 : (ob + 1) * 512],
                        w8v[:, :, 256 + ob * 128 : 256 + (ob + 1) * 128],
                        x8v[:, :, ts],
                        start=True, stop=True, perf_mode=DR,
                    )
                pv = ps.rearrange("p (a n) -> p a n", a=2)
                nc.vector.tensor_copy(out=k8v[:, :, ts], in_=pv)

            def emit_v(vc):
                # v chunk vc: key tiles 4vc..4vc+3 -> vT8 pair-layout, /16
                ps = psS.tile([128, 1024], F32, tag="s", name=f"vp{vc}")
                for h in range(4):
                    t = 4 * vc + h
                    nc.tensor.matmul(
                        ps[:, h * 256 : (h + 1) * 256],
                        x8v[:, :, t * 128 : (t + 1) * 128],
                        w8v[:, :, 512:768],
                        start=True, stop=True, perf_mode=DR,
                    )
                nc.vector.tensor_scalar_mul(
                    out=vT8[:, vc * 1024 : (vc + 1) * 1024],
                    in0=ps, scalar1=1.0 / 16.0,
                )

            emit_q(0)
            emit_k(0)

            # ---------------- attention ----------------
            og_tiles = {}

            def emit_proj(g):
                # both output-channel blocks of group g's proj back-to-back
                # (single psS slot lifetime -> less pool contention)
                qs = slice(g * QG, (g + 1) * QG)
                ps = psS.tile([128, 1024], F32, tag="s", name=f"pp{g}")
                og = og_tiles.pop(g)
                ogv = og.rearrange("p (a n) -> p a n", a=2)
                for pbk in range(2):
                    half = ps[:, pbk * QG : (pbk + 1) * QG]
                    nc.tensor.matmul(
                        half, wp8v[:, :, pbk * 128 : (pbk + 1) * 128], ogv,
                        start=True, stop=True, perf_mode=DR,
                    )
                for pbk in range(2):
                    half = ps[:, pbk * QG : (pbk + 1) * QG]
                    t1 = t1p.tile([128, QG], F32, tag="t1", name=f"t1_{g}_{pbk}")
                    nc.vector.scalar_tensor_tensor(
                        out=t1, in0=half, scalar=1.0 / 16.0,
                        in1=xqp_tiles[g][:, pbk * QG : (pbk + 1) * QG],
                        op0=ALU.mult, op1=ALU.add,
                    )
                    dq = nc.sync if pbk == 0 else nc.gpsimd
                    dq.dma_start(
                        out=out[pbk * 128 : (pbk + 1) * 128, qs], in_=t1
                    )
                xqp_tiles.pop(g)

            xqp_tiles = {}

            def emit_xqp(g):
                # residual + proj bias for group g, built on gpsimd (SBUF-only)
                xqp = xqpp.tile([128, 2 * QG], F32, tag="xqp", name=f"xqp{g}")
                for pbk in range(2):
                    nc.gpsimd.tensor_scalar_add(
                        out=xqp[:, pbk * QG : (pbk + 1) * QG],
                        in0=xq_sb[:, pbk * NQ + g * QG : pbk * NQ + (g + 1) * QG],
                        scalar1=pbt[:, pbk : pbk + 1],
                    )
                xqp_tiles[g] = xqp

            # Software-pipelined flat loop over all 64 pairs: the PE consumes
            # pair t-L (sums+PV) while the ACT engine exps pair t.
            L = 2
            NT = NGROUPS * NPAIR
            pts = [None] * NT
            sums_ps = None
            o_ps = None

            for t in range(NT + L):
                if t < NT:
                    g, tp = divmod(t, NPAIR)
                    qs = slice(g * QG, (g + 1) * QG)
                    sc = psS.tile([128, 1024], F32, tag="s", name=f"sc{t}")
                    for h in range(2):
                        kt = 2 * tp + h
                        nc.tensor.matmul(
                            sc[:, h * 512 : (h + 1) * 512],
                            k8v[:, :, kt * 128 : (kt + 1) * 128],
                            q8v[:, :, qs],
                            start=True, stop=True, perf_mode=DR,
                        )
                    pT = ptp.tile([128, 1024], F8, tag="pT", name=f"pT{t}")
                    # k8 is unscaled (16x): s_true = psum / (16*16). The -3
                    # bias keeps exp under fp8 max (448); it cancels in the
                    # softmax ratio.
                    nc.scalar.activation(
                        out=pT, in_=sc, func=AF.Exp, scale=1.0 / 256.0, bias=expb_t
                    )
                    pts[t] = pT

                    # production interleave, front-loaded into pipeline fill
                    if g == 0:
                        sched = {
                            0: [(emit_v, 0), (emit_k, 1)],
                            1: [(emit_k, 2), (emit_pbt,)],
                            2: [(emit_v, 1), (emit_k, 3)],
                            3: [(emit_v, 2)],
                            4: [(emit_k, 4)],
                            5: [(emit_v, 3)],
                            6: [(emit_k, 5)],
                            7: [(emit_v, 4)],
                            8: [(emit_k, 6)],
                            9: [(emit_v, 5)],
                            10: [(emit_k, 7)],
                            11: [(emit_v, 6)],
                            12: [(emit_v, 7)],
                            13: [(emit_q, 1)],
                        }.get(tp, [])
                        for fn, *args in sched:
                            fn(*args)
                    if g in (1, 2) and tp == 8:
                        emit_q(g + 1)

                if t >= L:
                    c = t - L
                    gc, tpc = divmod(c, NPAIR)
                    if tpc == 11:
                        emit_xqp(gc)
                    if tpc == 0:
                        sums_ps = psU.tile([1, QG], F32, tag="u", name=f"sums{gc}")
                        o_ps = [
                            psO.tile([128, QG], F32, tag="o", name=f"ops{gc}_{i}")
                            for i in range(2)
                        ]
                    pTv = pts[c].rearrange("p (a n) -> p a n", a=2)
                    pts[c] = None
                    # rowsums into [1, QG] (1-col stationary: no LDW cost) + PV
                    nc.tensor.matmul(
                        sums_ps, ones8v1, pTv,
                        start=(tpc == 0), stop=(tpc == NPAIR - 1), perf_mode=DR,
                    )
                    vv = vT8[:, tpc * 512 : (tpc + 1) * 512].rearrange(
                        "p (a n) -> p a n", a=2
                    )
                    for cbk in range(2):
                        nc.tensor.matmul(
                            o_ps[cbk],
                            vv[:, :, cbk * 128 : (cbk + 1) * 128],
                            pTv,
                            start=(tpc == 0), stop=(tpc == NPAIR - 1), perf_mode=DR,
                        )
                    if tpc == NPAIR - 1:
                        # 1/rowsums -> fp16 -> broadcast to 128 partitions via
                        # a tiny fp16 matmul; then normalize -> fp8 og
                        rb = rbp.tile([1, QG], F32, tag="rb", name=f"rb{gc}")
                        nc.vector.reciprocal_approx_fast(out=rb, in_=sums_ps)
                        rb16 = rbp.tile([1, QG], F16, tag="rb16", name=f"rb16{gc}")
                        nc.vector.tensor_copy(out=rb16, in_=rb)
                        rb_ps = psU.tile([128, QG], F32, tag="u", name=f"rbps{gc}")
                        nc.tensor.matmul(
                            rb_ps, ones16, rb16, start=True, stop=True
                        )
                        rb_sb = rbp.tile([128, QG], F32, tag="rbsb", name=f"rbsb{gc}")
                        nc.vector.tensor_copy(out=rb_sb, in_=rb_ps)
                        og = ogp.tile([128, 2 * QG], F8, tag="og", name=f"og{gc}")
                        if gc == NGROUPS - 1:
                            for qh in range(2):
                                for cbk in range(2):
                                    nc.vector.tensor_mul(
                                        out=og[:, cbk * QG + qh * 256 : cbk * QG + (qh + 1) * 256],
                                        in0=o_ps[cbk][:, qh * 256 : (qh + 1) * 256],
                                        in1=rb_sb[:, qh * 256 : (qh + 1) * 256],
                                    )
                        else:
                            for cbk in range(2):
                                nc.vector.tensor_mul(
                                    out=og[:, cbk * QG : (cbk + 1) * QG],
                                    in0=o_ps[cbk], in1=rb_sb,
                                )
                        og_tiles[gc] = og
                        if debug and gc == 0:
                            nc.sync.dma_start(out=dbg["d_og"][:, :], in_=og)
                            nc.scalar.dma_start(out=dbg["d_rb"][:, :], in_=rb)
                    elif tpc == 1 and gc > 0:
                        emit_proj(gc - 1)

            # final group's epilogue: split into query-halves so og/proj/stt/
            # DMA pipeline against each other
            gf = NGROUPS - 1
            og = og_tiles.pop(gf)
            ogv2 = og.rearrange("p (a n) -> p a n", a=2)
            dqs = [nc.sync, nc.gpsimd, nc.scalar, nc.sync]
            for qh in range(2):
                ps = psS.tile([128, 512], F32, tag="s", name=f"fp{qh}")
                for pbk in range(2):
                    half = ps[:, pbk * 256 : (pbk + 1) * 256]
                    nc.tensor.matmul(
                        half, wp8v[:, :, pbk * 128 : (pbk + 1) * 128],
                        ogv2[:, :, qh * 256 : (qh + 1) * 256],
                        start=True, stop=True, perf_mode=DR,
                    )
                    t1 = t1p.tile([128, 256], F32, tag="t1f", name=f"t1f{qh}_{pbk}")
                    nc.vector.scalar_tensor_tensor(
                        out=t1, in0=half, scalar=1.0 / 16.0,
                        in1=xqp_tiles[gf][:, pbk * QG + qh * 256 :
                                          pbk * QG + (qh + 1) * 256],
                        op0=ALU.mult, op1=ALU.add,
                    )
                    dqs[qh * 2 + pbk].dma_start(
                        out=out[pbk * 128 : (pbk + 1) * 128,
                                gf * QG + qh * 256 : gf * QG + (qh + 1) * 256],
                        in_=t1,
                    )

            if debug:
                nc.sync.dma_start(out=dbg["d_q8"][:, :], in_=q8)
                nc.sync.dma_start(out=dbg["d_k8"][:, :], in_=k8)
                nc.sync.dma_start(out=dbg["d_v8"][:, :], in_=vT8)
                nc.sync.dma_start(out=dbg["d_w8"][:, :], in_=w8)
                nc.scalar.dma_start(out=dbg["d_rstd"][:, :], in_=rstd)
                mvf = tmpp.tile([128, 4], F32, tag="mvf")
                nc.vector.tensor_copy(out=mvf, in_=cst)
                nc.scalar.dma_start(out=dbg["d_mv"][:, :], in_=mvf)

    nc.finalize()
    return nc


_NC_CACHE = {}


def _get_nc(debug=False):
    if debug not in _NC_CACHE:
        _NC_CACHE[debug] = _build_nc(debug)
    return _NC_CACHE[debug]


def _host_constants(qkv_w, qkv_b, proj_w, proj_b, gn_w, gn_b):
    """Pack weights into bf16 [128, NCBF] + f32 [128, NCF32] blocks."""
    # gn_w and the x16 fp8 scale fold into the bf16 weights; gn_b folds into
    # per-channel host bias vectors (cq, cv).
    wq16 = (qkv_w * gn_w[None, :]).T * 16.0      # [256, 768]
    wp16 = proj_w.T * 16.0                        # [256, 256]
    cbf = np.zeros((128, NCBF), np.float32)
    cbf[:, 0:768] = wq16[0:128]
    cbf[:, 768:1536] = wq16[128:256]
    cbf[:, OFF_WPROJ : OFF_WPROJ + 256] = wp16[0:128]
    cbf[:, OFF_WPROJ + 256 : OFF_WPROJ + 512] = wp16[128:256]

    cf = np.zeros((128, NCF32), np.float32)
    for c in range(128):
        g0 = c // 32
        cf[c, OFF_GRPAVG + g0 * 32 : OFF_GRPAVG + (g0 + 1) * 32] = 1.0 / 32.0
    wgnb = qkv_w @ gn_b                           # [768]
    cqv = qkv_b[0:256] + wgnb[0:256]
    cvv = qkv_b[512:768] + wgnb[512:768]
    for j in range(2):
        cf[:, OFF_CQ + j] = cqv[j * 128 : (j + 1) * 128]
        cf[:, OFF_CV + j] = cvv[j * 128 : (j + 1) * 128]
        cf[:, OFF_CP + j] = proj_b[j * 128 : (j + 1) * 128]
    return cbf.astype(ml_dtypes.bfloat16), cf


def _make_in_maps(x, gn_w, gn_b, qkv_w, qkv_b, proj_w, proj_b):
    x2d = np.asarray(x, np.float32).reshape(B, C, HW)
    cbf, cf = _host_constants(
        np.asarray(qkv_w, np.float32), np.asarray(qkv_b, np.float32),
        np.asarray(proj_w, np.float32), np.asarray(proj_b, np.float32),
        np.asarray(gn_w, np.float32), np.asarray(gn_b, np.float32),
    )
    in_maps = []
    for core in range(NCORES):
        b, qh = core // 2, core % 2
        q0 = qh * NQ
        xb = x2d[b]
        # own query half first; key-column permutation is harmless
        xp = np.concatenate([xb[:, q0 : q0 + NQ], xb[:, NQ - q0 : HW - q0]], axis=1)
        x8 = np.ascontiguousarray(
            xp.reshape(2, 128, HW).transpose(1, 0, 2).reshape(128, 2 * HW)
        ).astype(ml_dtypes.float8_e4m3fn)
        xqh = np.ascontiguousarray(xp[:, :NQ])
        in_maps.append({"x8in": x8, "xq": xqh, "cbf": cbf, "cf32": cf})
    return in_maps


def kernel(x, gn_w, gn_b, qkv_w, qkv_b, proj_w, proj_b):
    in_maps = _make_in_maps(x, gn_w, gn_b, qkv_w, qkv_b, proj_w, proj_b)
    res = run_bass_kernel_spmd(_get_nc(), in_maps, core_ids=list(range(NCORES)))

    out = np.empty((B, C, HW), np.float32)
    for core in range(NCORES):
        b, qh = core // 2, core % 2
        q0 = qh * NQ
        out[b][:, q0 : q0 + NQ] = res.results[core]["out"]
    return out.reshape(B, C, 64, 64)


def _run_traced(inputs):
    """Profiled run (trace=True); returns BassKernelResults."""
    in_maps = _make_in_maps(**inputs)
    return run_bass_kernel_spmd(
        _get_nc(), in_maps, core_ids=list(range(NCORES)), trace=True
    )


def _run_debug(inputs):
    in_maps = _make_in_maps(**inputs)
    return run_bass_kernel_spmd(
        _get_nc(debug=True), in_maps, core_ids=list(range(NCORES))
    )
